# revision 38
# baseline (speedup 1.0000x reference)
"""Causal self-attention on 8 Trainium2 NeuronCores.

Problem: B=2, T=2048, C=1024, 16 heads x 64 dim, fp32 in/out.

Sharding: tensor-parallel over heads x data-parallel over batch.
Each core owns one batch element (cores 0-3 -> b=0, 4-7 -> b=1) and a
group of 4 consecutive heads. Each core computes:
  - QKV projection for its 4 heads (qT/kT transposed, V natural)
  - causal attention for its 4 heads (scores kept transposed: ST[tk, tq])
  - partial output projection (its heads' rows of w_proj)
The host sums the 4 partial projections per batch and adds b_proj.

All matmul operands are bf16 (fp32 PSUM accumulation); partial outputs
are written as fp16. Engine budget per core (measured/modeled):
  PE  ~113us  (QKV 41, attention ST+PV 58, out-proj 14)   <- bottleneck
  ACT ~75us   (exp over causal score tiles, 1 el/cyc/lane @1.2GHz)
  DVE ~55us   (q/k bias copies, V bias, mask, softmax normalize, out copies)
Schedule: the attention inner loop is ACT-bound (exp 1147ns/tile vs
852ns of PE work), so QKV slabs 2-3 and all out-projection matmuls are
sliced into ~213ns "filler" steps and injected between attention tks to
keep the PE saturated. ACT runs exp ONLY (bias copies on DVE) so the
exp stream never stalls behind PE-dependent work. The two head-pairs
alternate per slab so one pair's softmax-normalize chain (reciprocal ->
partition_broadcast -> multiply) hides under the other pair's compute.

PSUM (8 banks): ST pair tiles [128,1024] x2 (4), psy accumulators
[128,512] x2 (2), filler/QKV/out-proj tiles [128,512] x2 (2).
"""

import collections

import numpy as np

B, T, C = 2, 2048, 1024
NH, DH = 16, 64
NCORES = 8
HPC = 4  # heads per core
P = 128
CK = C // P  # 8 contraction tiles over channels
NT = T // P  # 16 token tiles
SLAB = 512
NSL = T // SLAB  # 4 tq slabs

_CACHE = {}


def _build_program():
    from contextlib import ExitStack

    import concourse.bacc as bacc
    import concourse.bass as bass
    import concourse.tile as tile
    from concourse import mybir

    f32 = mybir.dt.float32
    f16 = mybir.dt.float16
    bf16 = mybir.dt.bfloat16
    AF = mybir.ActivationFunctionType

    nc = bacc.Bacc(
        "TRN2", target_bir_lowering=False, debug=False, num_devices=NCORES
    )

    xT = nc.dram_tensor("xT", [C, T], bf16, kind="ExternalInput").ap()
    wqk = nc.dram_tensor("wqk", [C, 4 * P], bf16, kind="ExternalInput").ap()
    wv = nc.dram_tensor("wv", [C, HPC * DH], bf16, kind="ExternalInput").ap()
    wo = nc.dram_tensor("wo", [HPC * DH, C], bf16, kind="ExternalInput").ap()
    bqk = nc.dram_tensor("bqk", [4, P], f32, kind="ExternalInput").ap()
    bv = nc.dram_tensor("bv", [HPC * DH], f32, kind="ExternalInput").ap()
    mask = nc.dram_tensor("mask", [P, P], bf16, kind="ExternalInput").ap()
    out = nc.dram_tensor("out", [T, C], f16, kind="ExternalOutput").ap()

    with tile.TileContext(nc) as tc, ExitStack() as ctx:
        const = ctx.enter_context(tc.tile_pool(name="const", bufs=1))
        # PSUM: st 2x[128,1024] (4 banks) + psy 2x[128,512] (2 banks)
        #       + fill 2x[128,512] (2 banks) = 8 banks
        ps = ctx.enter_context(tc.tile_pool(name="ps", bufs=2, space="PSUM"))
        expp = ctx.enter_context(tc.tile_pool(name="expp", bufs=4))
        rbp = ctx.enter_context(tc.tile_pool(name="rbp", bufs=2))
        outp = ctx.enter_context(tc.tile_pool(name="outp", bufs=3))

        x_ch = [
            const.tile([P, CK, SLAB], bf16, name=f"x_ch{c}") for c in range(NSL)
        ]
        wqk_sb = const.tile([P, CK, 4 * P], bf16, name="wqk_sb")
        wv_sb = const.tile([P, CK, HPC * DH], bf16, name="wv_sb")
        wo_sb = const.tile([P, 2, C], bf16, name="wo_sb")
        bqk_sb = const.tile([P, 4], f32, name="bqk_sb")
        bv_sb = const.tile([P, HPC, DH], f32, name="bv_sb")
        mask_sb = const.tile([P, P], bf16, name="mask_sb")
        # per-slab tiles so Tile's dependency tracking stays precise: a
        # reader of slab s must never wait on a later slab's writer
        v_sbs = [
            const.tile([P, 4, HPC, DH + 1], bf16, name=f"v_sb{s}")
            for s in range(NSL)
        ]
        qTs = [
            [const.tile([P, SLAB], bf16, name=f"qT{p}_{s}") for s in range(NSL)]
            for p in range(2)
        ]
        kTs = [
            [const.tile([P, SLAB], bf16, name=f"kT{p}_{s}") for s in range(NSL)]
            for p in range(2)
        ]
        yTs = [
            [const.tile([P, SLAB], bf16, name=f"yT{p}_{s}") for s in range(NSL)]
            for p in range(2)
        ]
        wrm = const.tile([1, 8], f32, name="wrm")

        # --- loads in exact need order with few dispatches (each dma_start
        #     costs ~640ns of dispatch on the Sync sequencer) ---
        wqkv = wqk.rearrange("(k p) n -> p k n", p=P)
        xTp = xT.rearrange("(k p) t -> p k t", p=P)
        nc.sync.dma_start(out=wqk_sb[:, :, 0:P], in_=wqkv[:, :, 0:P])
        nc.sync.dma_start(out=x_ch[0][:, 0:2, :], in_=xTp[:, 0:2, 0:SLAB])
        nc.sync.dma_start(out=wv_sb[:], in_=wv.rearrange("(k p) n -> p k n", p=P))
        nc.sync.dma_start(out=x_ch[0][:, 2:5, :], in_=xTp[:, 2:5, 0:SLAB])
        nc.sync.dma_start(out=wqk_sb[:, :, P:], in_=wqkv[:, :, P:])
        nc.sync.dma_start(out=x_ch[0][:, 5:, :], in_=xTp[:, 5:, 0:SLAB])
        nc.sync.dma_start(out=mask_sb[:], in_=mask)
        nc.sync.dma_start(out=bqk_sb[:], in_=bqk.rearrange("r p -> p r"))
        bv_bcast = bass.AP(
            tensor=bv.tensor,
            offset=bv.offset,
            ap=[[0, P], *bv.rearrange("(h d) -> h d", d=DH).ap],
        )
        nc.sync.dma_start(out=bv_sb[:], in_=bv_bcast)
        for c in range(1, NSL):
            nc.sync.dma_start(
                out=x_ch[c][:, 0:4, :], in_=xTp[:, 0:4, c * SLAB : (c + 1) * SLAB]
            )
            nc.sync.dma_start(
                out=x_ch[c][:, 4:, :], in_=xTp[:, 4:, c * SLAB : (c + 1) * SLAB]
            )
        nc.sync.dma_start(out=wo_sb[:], in_=wo.rearrange("(r p) n -> p r n", p=P))
        for s in range(NSL):
            nc.vector.memset(v_sbs[s][:, :, :, DH : DH + 1], 1.0)
        # dummy exp so the ACT table set loads during the QKV prologue
        nc.vector.memset(wrm[:], 0.0)
        nc.scalar.activation(out=wrm[:, 4:8], in_=wrm[:, 0:4], func=AF.Exp)
        # warm-up matmuls: keep the PE busy (and HAM un-throttled) while the
        # first x/wqk transfers are still in flight
        wsrc = const.tile([P, SLAB], bf16, name="wsrc")
        nc.vector.memset(wsrc[:], 0.0)
        for w in range(10):
            wps = ps.tile([P, SLAB], f32, name="wps", tag="fill", bufs=2)
            nc.tensor.matmul(
                wps[:], lhsT=wsrc[:, 0:P], rhs=wsrc[:], start=True, stop=True
            )

        # --- work-unit generators (each yield ~213ns of PE work) ---
        def qk_block_gen(s, blk):
            p_, qk = divmod(blk, 2)
            dst = (qTs if qk == 0 else kTs)[p_][s]
            ft = ps.tile([P, SLAB], f32, name="ftq", tag="fill", bufs=2)
            for k in range(CK):
                nc.tensor.matmul(
                    ft[:],
                    lhsT=wqk_sb[:, k, blk * P : (blk + 1) * P],
                    rhs=x_ch[s][:, k, :],
                    start=(k == 0),
                    stop=(k == CK - 1),
                )
                if k < CK - 1:
                    yield
            nc.vector.tensor_scalar_add(
                out=dst[:], in0=ft[:], scalar1=bqk_sb[:, blk : blk + 1]
            )

        def v_tt_gen(s, tt):
            ft = ps.tile([P, SLAB], f32, name="ftv", tag="fill", bufs=2)
            fv = ft[:, 0 : HPC * DH]
            for k in range(CK):
                nc.tensor.matmul(
                    fv,
                    lhsT=x_ch[s][:, k, tt * P : (tt + 1) * P],
                    rhs=wv_sb[:, k, :],
                    start=(k == 0),
                    stop=(k == CK - 1),
                )
                if k % 2 == 1 and k < CK - 1:
                    yield
            nc.vector.tensor_add(
                out=v_sbs[s][:, tt, :, 0:DH],
                in0=fv.rearrange("p (h d) -> p h d", d=DH),
                in1=bv_sb[:],
            )

        def outproj_gen(t):
            # p-major: both pair-0 matmuls run before any pair-1 matmul, so
            # a pending pair-1 normalize never blocks the first half.
            fts = [
                ps.tile([P, SLAB], f32, name=f"fto{ns}", tag="fill", bufs=2)
                for ns in range(2)
            ]
            for p_ in range(2):
                for ns in range(2):
                    nc.tensor.matmul(
                        fts[ns][:],
                        lhsT=yTs[p_][t // 4][:, (t % 4) * P : (t % 4 + 1) * P],
                        rhs=wo_sb[:, p_, ns * SLAB : (ns + 1) * SLAB],
                        start=(p_ == 0),
                        stop=(p_ == 1),
                    )
                    yield
            for ns in range(2):
                ob = outp.tile([P, SLAB], f16, name="ob", tag="ob", bufs=3)
                nc.vector.tensor_copy(out=ob[:], in_=fts[ns][:])
                nc.sync.dma_start(
                    out=out[t * P : (t + 1) * P, ns * SLAB : (ns + 1) * SLAB],
                    in_=ob[:],
                )

        def outproj_tail_gen(t):
            # Tail variant: at the tail every PSUM tag is free, so rotate
            # across st/fill/psy to keep 3 tiles in flight, and alternate
            # evacuation between ACT and DVE — the fill ring and the DVE
            # queue must not serialize the final projections.
            tag = ("st", "fill", "psy")[t % 3]
            if tag == "st":
                ft = ps.tile([P, 2 * SLAB], f32, name="ftt", tag="st", bufs=2)
                fts = [ft[:, 0:SLAB], ft[:, SLAB:]]
            else:
                fts = [
                    ps.tile([P, SLAB], f32, name=f"ftt{ns}", tag=tag, bufs=2)[:]
                    for ns in range(2)
                ]
            for p_ in range(2):
                for ns in range(2):
                    nc.tensor.matmul(
                        fts[ns],
                        lhsT=yTs[p_][t // 4][:, (t % 4) * P : (t % 4 + 1) * P],
                        rhs=wo_sb[:, p_, ns * SLAB : (ns + 1) * SLAB],
                        start=(p_ == 0),
                        stop=(p_ == 1),
                    )
                    yield
            ob = outp.tile([P, 2 * SLAB], f16, name="ob2", tag="ob2", bufs=3)
            if t % 2 == 0:
                nc.scalar.activation(out=ob[:, 0:SLAB], in_=fts[0], func=AF.Copy)
                nc.scalar.activation(out=ob[:, SLAB:], in_=fts[1], func=AF.Copy)
            else:
                nc.vector.tensor_copy(out=ob[:, 0:SLAB], in_=fts[0])
                nc.vector.tensor_copy(out=ob[:, SLAB:], in_=fts[1])
            nc.sync.dma_start(out=out[t * P : (t + 1) * P, :], in_=ob[:])

        fillers = collections.deque()

        def drain_fill(n):
            steps = 0
            while fillers and steps < n:
                try:
                    next(fillers[0])
                    steps += 1
                except StopIteration:
                    fillers.popleft()

        def qkv_dense0():
            # Dense slab-0 prologue covering only what att(0, p0) needs:
            # pair-0 q/k blocks + all V. k-major so every arriving x chunk
            # immediately feeds 6 matmuls. Pair-1 blocks arrive as the first
            # fillers inside att(0, p0).
            s = 0
            stt = ps.tile([P, 2 * SLAB], f32, name="qkd", tag="st", bufs=2)
            vts = [
                ps.tile([P, HPC * DH], f32, name=f"vtd{j}", tag=tg, bufs=2)
                for j, tg in enumerate(("psy", "psy", "fill", "fill"))
            ]
            for k in range(CK):
                for blk in range(2):
                    nc.tensor.matmul(
                        stt[:, blk * SLAB : (blk + 1) * SLAB],
                        lhsT=wqk_sb[:, k, blk * P : (blk + 1) * P],
                        rhs=x_ch[s][:, k, :],
                        start=(k == 0),
                        stop=(k == CK - 1),
                    )
                for tt in range(4):
                    nc.tensor.matmul(
                        vts[tt][:],
                        lhsT=x_ch[s][:, k, tt * P : (tt + 1) * P],
                        rhs=wv_sb[:, k, :],
                        start=(k == 0),
                        stop=(k == CK - 1),
                    )
            for blk in range(2):
                p_, qk = divmod(blk, 2)
                dst = (qTs if qk == 0 else kTs)[p_][s]
                nc.vector.tensor_scalar_add(
                    out=dst[:],
                    in0=stt[:, blk * SLAB : (blk + 1) * SLAB],
                    scalar1=bqk_sb[:, blk : blk + 1],
                )
            for tt in range(4):
                nc.vector.tensor_add(
                    out=v_sbs[s][:, tt, :, 0:DH],
                    in0=vts[tt][:].rearrange("p (h d) -> p h d", d=DH),
                    in1=bv_sb[:],
                )

        # --- attention for one (pair, slab): ST -> exp -> PV pipelined,
        #     filler steps injected after each tk's PV pair ---
        def att_slab(p, s):
            psy = [
                ps.tile([P, SLAB], f32, name=f"psy{hp}", tag="psy", bufs=2)
                for hp in range(2)
            ]
            ntk = 4 * s + 4  # tk tiles 0 .. 4s+3 (causal)

            def off_of(tk):
                diag_r = tk - 4 * s
                return diag_r * P if diag_r >= 0 else 0

            def st_pair(tk):
                off = off_of(tk)
                pp = ps.tile([P, 2 * SLAB], f32, name="pp", tag="st", bufs=2)
                for hp in range(2):
                    nc.tensor.matmul(
                        pp[:, hp * SLAB + off : (hp + 1) * SLAB],
                        lhsT=kTs[p][tk // 4][
                            hp * DH : (hp + 1) * DH, (tk % 4) * P : (tk % 4 + 1) * P
                        ],
                        rhs=qTs[p][s][hp * DH : (hp + 1) * DH, off:],
                        start=True,
                        stop=True,
                    )
                return pp

            def do_exp(tk):
                off = off_of(tk)
                pp = pend.pop(tk)
                ex = expp.tile([P, 2 * SLAB], bf16, name="ex", tag="ex")
                ppv = pp[:].rearrange("q (h n) -> q h n", h=2)[:, :, off:]
                exv = ex[:].rearrange("q (h n) -> q h n", h=2)[:, :, off:]
                nc.scalar.activation(
                    out=exv,
                    in_=ppv,
                    func=AF.Exp,
                    scale=float(1.0 / np.sqrt(DH)),
                )
                if tk - 4 * s >= 0:
                    for hp in range(2):
                        nc.vector.tensor_mul(
                            out=ex[:, hp * SLAB + off : hp * SLAB + off + P],
                            in0=ex[:, hp * SLAB + off : hp * SLAB + off + P],
                            in1=mask_sb[:],
                        )
                return ex

            pend = {0: st_pair(0)}
            if ntk > 1:
                pend[1] = st_pair(1)
            exd = {0: do_exp(0)}
            # fills here cover the previous slab's normalize latency while
            # exp(0) runs; the STs above were issued first so the ACT exp
            # stream is never left waiting behind filler matmuls
            drain_fill(3)
            for tk in range(ntk):
                off = off_of(tk)
                if tk + 2 < ntk:
                    pend[tk + 2] = st_pair(tk + 2)
                if tk + 1 < ntk:
                    exd[tk + 1] = do_exp(tk + 1)
                ex = exd.pop(tk)
                for hp in range(2):
                    nc.tensor.matmul(
                        psy[hp][0 : DH + 1, off:],
                        lhsT=v_sbs[tk // 4][:, tk % 4, 2 * p + hp, :],
                        rhs=ex[:, hp * SLAB + off : (hp + 1) * SLAB],
                        start=(tk == 0),
                        stop=(tk == ntk - 1),
                    )
                drain_fill((6, 2, 2, 3)[s])
            # softmax normalize: yT = psy[0:64] * (1 / psy[64]).
            # Issue stage-by-stage across both hp so the DVE never waits on
            # the gpsimd broadcasts mid-chain.
            sms, recs, rbs = [], [], []
            for hp in range(2):
                sm = rbp.tile([1, SLAB], f32, name="sm", tag="sm", bufs=2)
                if s == 3 and p == 1:
                    nc.scalar.activation(
                        out=sm[:], in_=psy[hp][DH : DH + 1, :], func=AF.Copy
                    )
                else:
                    nc.vector.tensor_copy(out=sm[:], in_=psy[hp][DH : DH + 1, :])
                sms.append(sm)
            for hp in range(2):
                rec = rbp.tile([1, SLAB], f32, name="rec", tag="rec", bufs=2)
                nc.vector.reciprocal_approx_fast(out=rec[:], in_=sms[hp][:])
                recs.append(rec)
            for hp in range(2):
                rb = rbp.tile([DH, SLAB], f32, name="rb", tag="rb", bufs=2)
                nc.gpsimd.partition_broadcast(out_ap=rb[:], in_ap=recs[hp][:])
                rbs.append(rb)
            for hp in range(2):
                nc.vector.tensor_mul(
                    out=yTs[p][s][hp * DH : (hp + 1) * DH, :],
                    in0=psy[hp][0:DH, :],
                    in1=rbs[hp][:],
                )

        # --- schedule ---
        qkv_dense0()
        fillers.append(qk_block_gen(0, 2))
        fillers.append(qk_block_gen(0, 3))
        for s in (1, 2, 3):
            for blk in range(4):
                fillers.append(qk_block_gen(s, blk))
            for tt in range(4):
                fillers.append(v_tt_gen(s, tt))
        # out-proj for slabs 0-1 feeds the filler stream; slabs 2-3 are
        # reserved for the tail, where they overlap the final softmax
        # normalize chain and keep the PE warm (attention is ACT-paced, so
        # idling the PE there is free — idling it at the tail is not)
        tail_fillers = collections.deque()
        for s in range(NSL):
            for p in range(2):
                att_slab(p, s)
            for tt in range(4):
                t = 4 * s + tt
                if s < 2:
                    fillers.append(outproj_gen(t))
                else:
                    tail_fillers.append(outproj_tail_gen(t))
        while fillers:
            drain_fill(100)
        fillers.extend(tail_fillers)
        tail_fillers.clear()
        while fillers:
            drain_fill(100)

    nc.compile()
    return nc


def get_program():
    if "nc" not in _CACHE:
        _CACHE["nc"] = _build_program()
    return _CACHE["nc"]


def make_core_inputs(x, w_attn, b_attn, w_proj, core):
    """Host-side shard preparation for one core."""
    import ml_dtypes

    bf16 = ml_dtypes.bfloat16
    b = core // 4
    g = core % 4
    heads = [4 * g + i for i in range(HPC)]

    xT = np.ascontiguousarray(np.asarray(x[b], np.float32).T.astype(bf16))

    def qcols(h):
        return w_attn[:, h * DH : (h + 1) * DH]

    def kcols(h):
        return w_attn[:, C + h * DH : C + (h + 1) * DH]

    def vcols(h):
        return w_attn[:, 2 * C + h * DH : 2 * C + (h + 1) * DH]

    h0, h1, h2, h3 = heads
    wqk = np.ascontiguousarray(
        np.concatenate(
            [qcols(h0), qcols(h1), kcols(h0), kcols(h1),
             qcols(h2), qcols(h3), kcols(h2), kcols(h3)],
            axis=1,
        ).astype(bf16)
    )
    wv = np.ascontiguousarray(
        np.concatenate([vcols(h) for h in heads], axis=1).astype(bf16)
    )
    bqk = np.stack(
        [
            np.concatenate([b_attn[h0 * DH : (h0 + 1) * DH], b_attn[h1 * DH : (h1 + 1) * DH]]),
            np.concatenate([b_attn[C + h0 * DH : C + (h0 + 1) * DH], b_attn[C + h1 * DH : C + (h1 + 1) * DH]]),
            np.concatenate([b_attn[h2 * DH : (h2 + 1) * DH], b_attn[h3 * DH : (h3 + 1) * DH]]),
            np.concatenate([b_attn[C + h2 * DH : C + (h2 + 1) * DH], b_attn[C + h3 * DH : C + (h3 + 1) * DH]]),
        ]
    ).astype(np.float32)
    bv = np.concatenate(
        [b_attn[2 * C + h * DH : 2 * C + (h + 1) * DH] for h in heads]
    ).astype(np.float32)
    wo = np.ascontiguousarray(
        w_proj[heads[0] * DH : (heads[-1] + 1) * DH, :].astype(bf16)
    )
    mask = np.triu(np.ones((P, P))).astype(bf16)
    return {
        "xT": xT,
        "wqk": wqk,
        "wv": wv,
        "wo": wo,
        "bqk": np.ascontiguousarray(bqk),
        "bv": np.ascontiguousarray(bv),
        "mask": mask,
    }


def kernel(x, w_attn, b_attn, w_proj, b_proj):
    from concourse.bass_utils import run_bass_kernel_spmd

    x = np.asarray(x, np.float32)
    w_attn = np.asarray(w_attn, np.float32)
    b_attn = np.asarray(b_attn, np.float32)
    w_proj = np.asarray(w_proj, np.float32)
    b_proj = np.asarray(b_proj, np.float32)

    nc = get_program()
    in_maps = [
        make_core_inputs(x, w_attn, b_attn, w_proj, core) for core in range(NCORES)
    ]
    res = run_bass_kernel_spmd(nc, in_maps, core_ids=list(range(NCORES)))
    outs = [np.asarray(m["out"], np.float32) for m in res.results]

    y = np.empty((B, T, C), np.float32)
    for b in range(B):
        y[b] = outs[4 * b] + outs[4 * b + 1] + outs[4 * b + 2] + outs[4 * b + 3]
        y[b] += b_proj[None, :]
    return y
